# revision 1
# baseline (speedup 1.0000x reference)
"""DiffLogic 3-layer network on 8 Trainium2 NeuronCores.

Strategy (data-parallel over batch, per spec hint):
  - Each core gets 512 of the 4096 batch rows. Weights/indices replicated.
  - Activations kept feature-major ([features, batch] fp16) so the
    per-layer feature gathers become row gathers served by gpsimd dma_gather
    (random 1KB rows from DRAM -> SBUF partitions).
  - Layer output rows are permuted (host-side index rewiring) so each
    layer's a-operand gather reads DRAM rows in ascending order (HBM
    locality); intermediate h tensors are stored partition-major so the
    h writes are 16KB-contiguous per partition.
  - The soft-logic mixture out = c0 + c1*a + c2*b + c3*a*b runs on
    DVE (tensor_scalar / tensor_tensor) + ACT (affine) with per-partition
    coefficient scalars.
  - Per-row coefficients c_k = softmax(w) @ OP_COEF are computed on-device
    from exp(w) partial sums (the OP_COEF columns are small-integer
    patterns, so each c_k is a signed subset sum of exp(w_j)).
  - The final group-sum (10240 -> 10 groups of 1024) is a PE matmul with
    one-hot column stationaries accumulating into one PSUM tile; the L3
    row permutation stays within groups so group sums are unchanged.
"""

import numpy as np

# ---- problem constants (hardcoded per contract) ----
B, D0, D1, D2, D3 = 4096, 1024, 8192, 8192, 10240
NCORES = 8
BS = B // NCORES  # 512 batch rows per core
K = 10
TAU = 30.0

LAYERS = [
    # (n_out, n_src)
    (D1, D0),
    (D2, D1),
    (D3, D2),
]
NCH = [o // 128 for o, _ in LAYERS]  # [64, 64, 80]
NCH_TOT = sum(NCH)  # 208
CH_OFF = [0, NCH[0], NCH[0] + NCH[1]]  # global chunk offsets
GRP = 8  # chunks per dma_gather (8*128 = 1024 rows per gather)
ACT_T1_MOD = 3  # chunks with lc % 8 < this run the t1 affine on ACT

_nc_cache = {}


def _build_nc(repeat=1, act_t1_mod=ACT_T1_MOD, tt_group=4, l3_double_mm=False, l3_matmul_all=True, grp=GRP, nq=2):
    from concourse import bacc, bass, mybir
    from concourse.tile import TileContext

    f16 = mybir.dt.float16
    f32 = mybir.dt.float32
    i16 = mybir.dt.int16
    Alu = mybir.AluOpType
    Act = mybir.ActivationFunctionType

    nc = bacc.Bacc(None, target_bir_lowering=False, num_swdge_queues=nq)

    # ---- I/O ----
    xT = nc.dram_tensor("xT", [D0, BS], f16, kind="ExternalInput")
    wall = nc.dram_tensor("wall", [128, 16, NCH_TOT], f32, kind="ExternalInput")
    idx_in = []
    for li, (o, _) in enumerate(LAYERS):
        ia = nc.dram_tensor(f"ia{li}", [128, o // 16], i16, kind="ExternalInput")
        ib = nc.dram_tensor(f"ib{li}", [128, o // 16], i16, kind="ExternalInput")
        idx_in.append((ia, ib))
    sel_in = nc.dram_tensor("sel", [128, K * K], f16, kind="ExternalInput")
    out_d = nc.dram_tensor("out", [K, BS], f32, kind="ExternalOutput")

    # intermediate activations, partition-major: h[p, c, b] = row (c*128+p)
    h_d = [
        nc.dram_tensor("h1", [128, NCH[0], BS], f16),
        nc.dram_tensor("h2", [128, NCH[1], BS], f16),
    ]
    # gather sources as flat [rows, BS] views
    src_ap = [
        lambda: xT[:],
        lambda: h_d[0][:].rearrange("p c b -> (p c) b"),
        lambda: h_d[1][:].rearrange("p c b -> (p c) b"),
    ]

    with TileContext(nc) as tc:
      for _rep in range(repeat):
        with (
            tc.tile_pool(name="pers", bufs=1) as pers,
            tc.tile_pool(name="psum", bufs=1, space="PSUM") as psump,
        ):
            # ---------- coefficients ----------
            ck = [pers.tile([128, NCH_TOT], f32, name=f"ck{k}") for k in range(4)]
            sel_t = pers.tile([128, K * K], f16)
            nc.sync.dma_start(out=sel_t[:], in_=sel_in[:])

            with tc.tile_pool(name="coef_tmp", bufs=1) as ctp:
                w_t = ctp.tile([128, 16, NCH_TOT], f32)
                nc.sync.dma_start(out=w_t[:], in_=wall[:])
                E = ctp.tile([128, 16, NCH_TOT], f32)
                nc.scalar.activation(out=E[:], in_=w_t[:], func=Act.Exp)

                def Ej(j):
                    return E[:, j : j + 1, :]

                P2 = ctp.tile([128, 8, NCH_TOT], f32)
                for j2 in range(8):
                    nc.vector.tensor_tensor(
                        out=P2[:, j2 : j2 + 1, :],
                        in0=Ej(2 * j2),
                        in1=Ej(2 * j2 + 1),
                        op=Alu.add,
                    )

                def P2j(j):
                    return P2[:, j : j + 1, :]

                Q = ctp.tile([128, 4, NCH_TOT], f32)
                for q in range(4):
                    nc.vector.tensor_tensor(
                        out=Q[:, q : q + 1, :],
                        in0=P2j(2 * q),
                        in1=P2j(2 * q + 1),
                        op=Alu.add,
                    )
                R = ctp.tile([128, 2, NCH_TOT], f32)
                for r in range(2):
                    nc.vector.tensor_tensor(
                        out=R[:, r : r + 1, :],
                        in0=Q[:, 2 * r : 2 * r + 1, :],
                        in1=Q[:, 2 * r + 1 : 2 * r + 2, :],
                        op=Alu.add,
                    )
                den = ctp.tile([128, 1, NCH_TOT], f32)
                nc.vector.tensor_tensor(
                    out=den[:], in0=R[:, 0:1, :], in1=R[:, 1:2, :], op=Alu.add
                )
                rden = ctp.tile([128, 1, NCH_TOT], f32)
                nc.vector.reciprocal(out=rden[:], in_=den[:])

                # num0 = R[1] (= sum_{j>=8} E_j)
                # num2 = Q1 - Q2
                num2 = ctp.tile([128, 1, NCH_TOT], f32)
                nc.vector.tensor_tensor(
                    out=num2[:], in0=Q[:, 1:2, :], in1=Q[:, 2:3, :], op=Alu.subtract
                )
                # num1 = (P2[1] + P2[3]) - (P2[4] + P2[6])
                num1 = ctp.tile([128, 1, NCH_TOT], f32)
                tA = ctp.tile([128, 1, NCH_TOT], f32)
                nc.vector.tensor_tensor(out=tA[:], in0=P2j(1), in1=P2j(3), op=Alu.add)
                tB = ctp.tile([128, 1, NCH_TOT], f32)
                nc.vector.tensor_tensor(out=tB[:], in0=P2j(4), in1=P2j(6), op=Alu.add)
                nc.vector.tensor_tensor(
                    out=num1[:], in0=tA[:], in1=tB[:], op=Alu.subtract
                )
                # num3 = (E1-E14) - (E2+E4) - (P2[3]+E6) + (P2[4]+E9) + (E11+E13)
                num3 = ctp.tile([128, 1, NCH_TOT], f32)
                s1 = ctp.tile([128, 1, NCH_TOT], f32)
                nc.vector.tensor_tensor(out=s1[:], in0=Ej(1), in1=Ej(14), op=Alu.subtract)
                s2 = ctp.tile([128, 1, NCH_TOT], f32)
                nc.vector.tensor_tensor(out=s2[:], in0=Ej(2), in1=Ej(4), op=Alu.add)
                s3 = ctp.tile([128, 1, NCH_TOT], f32)
                nc.vector.tensor_tensor(out=s3[:], in0=P2j(3), in1=Ej(6), op=Alu.add)
                s4 = ctp.tile([128, 1, NCH_TOT], f32)
                nc.vector.tensor_tensor(out=s4[:], in0=P2j(4), in1=Ej(9), op=Alu.add)
                s5 = ctp.tile([128, 1, NCH_TOT], f32)
                nc.vector.tensor_tensor(out=s5[:], in0=Ej(11), in1=Ej(13), op=Alu.add)
                nc.vector.tensor_tensor(out=num3[:], in0=s1[:], in1=s2[:], op=Alu.subtract)
                nc.vector.tensor_tensor(out=num3[:], in0=num3[:], in1=s3[:], op=Alu.subtract)
                nc.vector.tensor_tensor(out=num3[:], in0=num3[:], in1=s4[:], op=Alu.add)
                nc.vector.tensor_tensor(out=num3[:], in0=num3[:], in1=s5[:], op=Alu.add)

                for k, num in enumerate([R[:, 1:2, :], num1[:], num2[:], num3[:]]):
                    nc.vector.tensor_tensor(
                        out=ck[k][:].rearrange("p (o c) -> p o c", o=1),
                        in0=num,
                        in1=rden[:],
                        op=Alu.mult,
                    )

            # ---------- L3 matmul-reduction stationaries ----------
            if l3_matmul_all:
                nch3 = NCH[2]
                selk = []
                for k in range(4):
                    sk = pers.tile([128, nch3 * K], f16, name=f"selk{k}")
                    nc.vector.memset(sk[:], 0)
                    # dst col for chunk c=8q+r is 10c + c//8 = 81q + 10r
                    skap = sk[:]
                    dst = bass.AP(
                        skap.tensor, skap.offset,
                        [list(skap.ap[0]), [81, 10], [K, 8]],
                    )
                    ckap = ck[k][:]
                    src = bass.AP(
                        ckap.tensor, ckap.offset + CH_OFF[2],
                        [list(ckap.ap[0]), [8, 10], [1, 8]],
                    )
                    nc.vector.tensor_copy(out=dst, in_=src)
                    selk.append(sk)
                ones_t = pers.tile([128, BS], f16)
                nc.vector.memset(ones_t[:], 1.0)

            # ---------- main layers ----------
            psum_out = psump.tile([K, BS], f32, space="PSUM")
            with (
                tc.tile_pool(name="idxp", bufs=2) as idxp,
                tc.tile_pool(name="gath", bufs=3) as gath,
                tc.tile_pool(name="outp", bufs=3) as outp,
                tc.tile_pool(name="tmp", bufs=3) as tmp,
            ):
                for li, (o, n_src) in enumerate(LAYERS):
                    nch = NCH[li]
                    ia_t = idxp.tile([128, o // 16], i16, tag="ia")
                    nc.sync.dma_start(out=ia_t[:], in_=idx_in[li][0][:])
                    ib_t = idxp.tile([128, o // 16], i16, tag="ib")
                    nc.sync.dma_start(out=ib_t[:], in_=idx_in[li][1][:])

                    for g in range(nch // grp):
                        gA = gath.tile([128, grp, BS], f16, tag="gA")
                        nc.gpsimd.dma_gather(
                            out_ap=gA[:],
                            in_ap=src_ap[li](),
                            idxs_ap=ia_t[:, g * grp * 8 : (g + 1) * grp * 8],
                            num_idxs=grp * 128,
                            num_idxs_reg=grp * 128,
                            elem_size=BS,
                            single_packet=False,
                            queue_num=(2 * g) % nq,
                        )
                        gB = gath.tile([128, grp, BS], f16, tag="gB")
                        nc.gpsimd.dma_gather(
                            out_ap=gB[:],
                            in_ap=src_ap[li](),
                            idxs_ap=ib_t[:, g * grp * 8 : (g + 1) * grp * 8],
                            num_idxs=grp * 128,
                            num_idxs_reg=grp * 128,
                            elem_size=BS,
                            single_packet=False,
                            queue_num=(2 * g + 1) % nq,
                        )
                        ho = outp.tile([128, grp, BS], f16, tag="ho")
                        TG = tt_group
                        if li == 2 and l3_matmul_all:
                            for cg in range(grp // TG):
                                abg = tmp.tile([128, TG, BS], f16, tag="t2")
                                nc.vector.tensor_tensor(
                                    out=abg[:],
                                    in0=gA[:, cg * TG : (cg + 1) * TG, :],
                                    in1=gB[:, cg * TG : (cg + 1) * TG, :],
                                    op=Alu.mult,
                                )
                                for c4 in range(TG):
                                    c = cg * TG + c4
                                    lc = g * grp + c
                                    sl = slice(lc * K, (lc + 1) * K)
                                    first = lc == 0
                                    last = lc == NCH[2] - 1
                                    nc.tensor.matmul(
                                        out=psum_out[:], lhsT=selk[1][:, sl],
                                        rhs=gA[:, c, :], start=first, stop=False,
                                    )
                                    nc.tensor.matmul(
                                        out=psum_out[:], lhsT=selk[2][:, sl],
                                        rhs=gB[:, c, :], start=False, stop=False,
                                    )
                                    nc.tensor.matmul(
                                        out=psum_out[:], lhsT=selk[3][:, sl],
                                        rhs=abg[:, c4, :], start=False, stop=False,
                                    )
                                    nc.tensor.matmul(
                                        out=psum_out[:], lhsT=selk[0][:, sl],
                                        rhs=ones_t[:], start=False, stop=last,
                                    )
                            continue
                        for cg in range(grp // TG):
                            t1g = tmp.tile([128, TG, BS], f16, tag="t1")
                            t3g = tmp.tile([128, TG, BS], f16, tag="t3")
                            for c4 in range(TG):
                                c = cg * TG + c4
                                lc = g * grp + c  # layer-local chunk
                                gc = CH_OFF[li] + lc  # global chunk
                                b = gB[:, c, :]
                                # t1 = c3*b + c1
                                if lc % 8 < act_t1_mod:
                                    nc.scalar.activation(
                                        out=t1g[:, c4, :],
                                        in_=b,
                                        func=Act.Identity,
                                        scale=ck[3][:, gc : gc + 1],
                                        bias=ck[1][:, gc : gc + 1],
                                    )
                                else:
                                    nc.vector.tensor_scalar(
                                        out=t1g[:, c4, :],
                                        in0=b,
                                        scalar1=ck[3][:, gc : gc + 1],
                                        scalar2=ck[1][:, gc : gc + 1],
                                        op0=Alu.mult,
                                        op1=Alu.add,
                                    )
                                # t3 = c2*b + c0   (ACT affine)
                                nc.scalar.activation(
                                    out=t3g[:, c4, :],
                                    in_=b,
                                    func=Act.Identity,
                                    scale=ck[2][:, gc : gc + 1],
                                    bias=ck[0][:, gc : gc + 1],
                                )
                            # t2g = t1g * a (grouped)
                            t2g = tmp.tile([128, TG, BS], f16, tag="t2")
                            nc.vector.tensor_tensor(
                                out=t2g[:],
                                in0=t1g[:],
                                in1=gA[:, cg * TG : (cg + 1) * TG, :],
                                op=Alu.mult,
                            )
                            if li == 2 and l3_double_mm:
                                for c4 in range(TG):
                                    lc = g * grp + cg * TG + c4
                                    grp_i = lc // 8
                                    sl = sel_t[:, grp_i * K : (grp_i + 1) * K]
                                    nc.tensor.matmul(
                                        out=psum_out[:], lhsT=sl, rhs=t2g[:, c4, :],
                                        start=(lc == 0), stop=False,
                                    )
                                    nc.tensor.matmul(
                                        out=psum_out[:], lhsT=sl, rhs=t3g[:, c4, :],
                                        start=False, stop=(lc == NCH[2] - 1),
                                    )
                            else:
                                nc.vector.tensor_tensor(
                                    out=ho[:, cg * TG : (cg + 1) * TG, :],
                                    in0=t2g[:],
                                    in1=t3g[:],
                                    op=Alu.add,
                                )
                        if li < 2:
                            nc.sync.dma_start(
                                out=h_d[li][:, g * grp : (g + 1) * grp, :],
                                in_=ho[:],
                            )
                        elif not l3_double_mm:
                            for c in range(grp):
                                lc = g * grp + c
                                grp_i = lc // 8  # 1024 rows = 8 chunks per group
                                nc.tensor.matmul(
                                    out=psum_out[:],
                                    lhsT=sel_t[:, grp_i * K : (grp_i + 1) * K],
                                    rhs=ho[:, c, :],
                                    start=(lc == 0),
                                    stop=(lc == nch - 1),
                                )

            out_sb = pers.tile([K, BS], f32)
            nc.scalar.activation(
                out=out_sb[:], in_=psum_out[:], func=Act.Copy, scale=1.0 / TAU
            )
            nc.sync.dma_start(out=out_d[:], in_=out_sb[:])

    nc.compile()
    return nc


def _wrap_idx(idx: np.ndarray) -> np.ndarray:
    """int16 index layout for dma_gather: wrapped in 16 partitions,
    replicated to 128 partitions (8 gpsimd cores)."""
    n = idx.shape[0]
    blk = idx.astype(np.int16).reshape(n // 16, 16).T  # [16, n/16]
    return np.ascontiguousarray(np.tile(blk, (8, 1)))  # [128, n/16]


def _prep_shared(w1, w2, w3, idx_a1, idx_b1, idx_a2, idx_b2, idx_a3, idx_b3):
    """Host-side layout prep: per-layer output-row permutation sigma (sorting
    the a-gather), source-row remap pi into the partition-major stored
    layout, weight tile layout, and index wrapping."""
    ws = (w1, w2, w3)
    ias = (idx_a1, idx_a2, idx_a3)
    ibs = (idx_b1, idx_b2, idx_b3)

    shared = {}
    wall_parts = []
    pi_prev = None  # original source row -> stored virtual row
    for li in range(3):
        o = LAYERS[li][0]
        nch = NCH[li]
        ia = ias[li].astype(np.int64)
        ib = ibs[li].astype(np.int64)
        if pi_prev is not None:
            ia = pi_prev[ia]
            ib = pi_prev[ib]
        if li < 2:
            sigma = np.argsort(ia, kind="stable")
        else:
            # keep group structure: sort within each block of 1024 rows
            sigma = np.concatenate(
                [g * 1024 + np.argsort(ia[g * 1024 : (g + 1) * 1024], kind="stable")
                 for g in range(K)]
            )
        ia_s = ia[sigma]
        ib_s = ib[sigma]
        w_s = ws[li].astype(np.float32)[sigma]
        wall_parts.append(
            np.ascontiguousarray(w_s.reshape(nch, 128, 16).transpose(1, 2, 0))
        )
        shared[f"ia{li}"] = _wrap_idx(ia_s)
        shared[f"ib{li}"] = _wrap_idx(ib_s)
        if li < 2:
            inv = np.empty(o, np.int64)
            inv[sigma] = np.arange(o)
            pi_prev = (inv % 128) * nch + inv // 128

    shared["wall"] = np.ascontiguousarray(np.concatenate(wall_parts, axis=2))

    sel = np.zeros((128, K * K), np.float16)
    for g in range(K):
        sel[:, g * K + g] = 1.0
    shared["sel"] = sel
    return shared


def make_in_maps(x, **shared_inputs):
    shared = _prep_shared(**shared_inputs)
    in_maps = []
    for c in range(NCORES):
        xs = x[c * BS : (c + 1) * BS].astype(np.float16)  # [512, 1024]
        xT = np.ascontiguousarray(xs.T)  # [1024, 512]
        in_maps.append({"xT": xT, **shared})
    return in_maps


def get_nc(repeat=1, **opts):
    key = (repeat, tuple(sorted(opts.items())))
    if key not in _nc_cache:
        _nc_cache[key] = _build_nc(repeat, **opts)
    return _nc_cache[key]


def kernel(
    x, w1, w2, w3, idx_a1, idx_b1, idx_a2, idx_b2, idx_a3, idx_b3
) -> np.ndarray:
    from concourse.bass_utils import run_bass_kernel_spmd

    nc = get_nc()
    in_maps = make_in_maps(
        np.asarray(x),
        w1=np.asarray(w1),
        w2=np.asarray(w2),
        w3=np.asarray(w3),
        idx_a1=np.asarray(idx_a1),
        idx_b1=np.asarray(idx_b1),
        idx_a2=np.asarray(idx_a2),
        idx_b2=np.asarray(idx_b2),
        idx_a3=np.asarray(idx_a3),
        idx_b3=np.asarray(idx_b3),
    )
    res = run_bass_kernel_spmd(nc, in_maps, core_ids=list(range(NCORES)))
    out = np.empty((B, K), np.float32)
    for c in range(NCORES):
        out[c * BS : (c + 1) * BS] = res.results[c]["out"].T
    return out

